# revision 11
# baseline (speedup 1.0000x reference)
"""Trainium2 Bass kernel for nn_FCNNaccBaseline (conv1d x3 + BN + NALU x2 + linear).

Sharding: pure data parallelism over batch B=128 across 8 cores (16 samples each).
BatchNorm (training-mode) batch stats are computed as per-channel (sum, sumsq)
via one-pass DVE bn_stats per conv-output chunk, combined exactly on-chip, and
AllReduce'd across the 8 cores (3 tiny collectives, one per conv layer).

Per-core dataflow (all activations bf16 [128ch, L] layout, fp32 PSUM/stats):
  A) conv1 (K=8 matmul over taps, rhs = overlapping-window DMA from host-padded
     input) -> y1 tiles resident in SBUF; bn_stats.
  B) AllReduce L1 stats; bn1+relu applied in place (y1 -> x1); conv2 as
     5-tap x 2-outgroup shifted matmuls; bn_stats on PSUM; y2 spilled to HBM
     as bf16 (the only intermediate too big for SBUF).
  C) AllReduce L2 stats; stream y2 back, bn2+relu -> x2 (full-sample buffer,
     zero halos); conv3 as 3-tap x 2-ktile matmuls; y3 written into the SBUF
     slots x1 vacated; bn_stats.
  D) AllReduce L3 stats; bn3+relu with fused per-partition accumulation
     (accum_out) -> per-sample channel means (feat) with no extra reduce pass.
  E) NALU x2 + final linear entirely in fp32 on-chip; output [16,1] per core.

Conv biases are dropped on purpose: training-mode BN subtracts the batch mean,
which cancels any per-channel additive bias exactly.
"""

import sys

sys.path.insert(0, "/opt/trn_rl_repo")

import numpy as np
import ml_dtypes

from concourse import bacc, bass, mybir, tile
from concourse import bass_utils

F32 = mybir.dt.float32
BF16 = mybir.dt.bfloat16
AF = mybir.ActivationFunctionType
ALU = mybir.AluOpType

NCORES = 8
B = 128
L_IN = 4096
BN_EPS = 1e-5
NALU_EPS = 1e-10


def _chunks(lout):
    ch = [(i * 512, 512) for i in range(lout // 512)]
    if lout % 512:
        ch.append((lout - lout % 512, lout % 512))
    return ch


def build_nc(ncores=NCORES, n_loc=B // NCORES, l_in=L_IN, enable_asserts=False,
             stop_after=None):
    """Emit the per-core Bass/Tile program. Returns the compiled Bacc.

    stop_after in {"A","AR1","B","AR2","C","D"} truncates the program after
    that phase and DMAs a debug snapshot into the extra "dbg" output.
    """
    lout = l_in + 1          # conv1: pad=4, k=8 -> L+1; conv2/conv3 preserve it
    w = l_in + 8             # padded width of activation buffers
    CH = _chunks(lout)
    nch = len(CH)
    ntot = ncores * n_loc * lout  # global BN count per channel
    groups = [0, 1]

    nc = bacc.Bacc("TRN2", target_bir_lowering=False, debug=False,
                   enable_asserts=enable_asserts, num_devices=ncores)

    # ---- DRAM I/O (per core) ----
    xpad = nc.dram_tensor("xpad", [n_loc, w], BF16, kind="ExternalInput")
    w1t = nc.dram_tensor("w1t", [8, 128], BF16, kind="ExternalInput")
    w2t = nc.dram_tensor("w2t", [128, 10 * 128], BF16, kind="ExternalInput")
    w3t = nc.dram_tensor("w3t", [128, 6 * 128], BF16, kind="ExternalInput")
    bn1g = nc.dram_tensor("bn1g", [128, 1], F32, kind="ExternalInput")
    bn1b = nc.dram_tensor("bn1b", [128, 1], F32, kind="ExternalInput")
    bn2g = nc.dram_tensor("bn2g", [128, 2], F32, kind="ExternalInput")
    bn2b = nc.dram_tensor("bn2b", [128, 2], F32, kind="ExternalInput")
    bn3g = nc.dram_tensor("bn3g", [128, 1], F32, kind="ExternalInput")
    bn3b = nc.dram_tensor("bn3b", [128, 1], F32, kind="ExternalInput")
    n1w = nc.dram_tensor("n1w", [128, 128], F32, kind="ExternalInput")
    n1g = nc.dram_tensor("n1g", [128, 128], F32, kind="ExternalInput")
    n2w = nc.dram_tensor("n2w", [128, 16], F32, kind="ExternalInput")
    n2g = nc.dram_tensor("n2g", [128, 16], F32, kind="ExternalInput")
    fw = nc.dram_tensor("fw", [16, 1], F32, kind="ExternalInput")
    fbt = nc.dram_tensor("fbt", [16, 1], F32, kind="ExternalInput")
    outd = nc.dram_tensor("out", [n_loc, 1], F32, kind="ExternalOutput")
    dbg = (nc.dram_tensor("dbg", [128, 64], F32, kind="ExternalOutput")
           if stop_after else None)

    def win_ap(i, l0, n, parts):
        """Overlapping-window DRAM AP: row k of [parts, n] = xpad[i, l0+k : l0+k+n]."""
        a = xpad.ap()[i:i + 1, l0:l0 + n]
        a = a.copy()
        a.ap = mybir.VecI64Pair([[1, parts], [1, n]])
        return a

    with tile.TileContext(nc) as tc:
        with (
            tc.tile_pool(name="const", bufs=1) as cst,
            tc.tile_pool(name="big", bufs=n_loc + 1) as bigp,
            tc.tile_pool(name="x2", bufs=2) as x2p,
            tc.tile_pool(name="c1rhs", bufs=4) as c1p,
            tc.tile_pool(name="spo", bufs=6) as spop,
            tc.tile_pool(name="spi", bufs=6) as spip,
            tc.tile_pool(name="stats", bufs=2) as stp,
            tc.tile_pool(name="cmb", bufs=2) as cmbp,
            tc.tile_pool(name="dump", bufs=2) as dmpp,
            tc.tile_pool(name="accp", bufs=2) as accp,
            tc.tile_pool(name="small", bufs=1) as sml,
            tc.tile_pool(name="mm", bufs=6, space="PSUM") as mmp,
            tc.tile_pool(name="mm1", bufs=2, space="PSUM") as mm1p,
            tc.tile_pool(name="dram", bufs=1, space="DRAM") as drp,
        ):
            # ---- constants into SBUF ----
            w1sb = cst.tile([8, 128], BF16)
            nc.sync.dma_start(w1sb[:], w1t.ap())
            w2sb = cst.tile([128, 10 * 128], BF16)
            nc.sync.dma_start(w2sb[:], w2t.ap())
            w3sb = cst.tile([128, 6 * 128], BF16)
            nc.sync.dma_start(w3sb[:], w3t.ap())
            g1sb = cst.tile([128, 1], F32); nc.sync.dma_start(g1sb[:], bn1g.ap())
            b1sb = cst.tile([128, 1], F32); nc.sync.dma_start(b1sb[:], bn1b.ap())
            g2sb = cst.tile([128, 2], F32); nc.sync.dma_start(g2sb[:], bn2g.ap())
            b2sb = cst.tile([128, 2], F32); nc.sync.dma_start(b2sb[:], bn2b.ap())
            g3sb = cst.tile([128, 1], F32); nc.sync.dma_start(g3sb[:], bn3g.ap())
            b3sb = cst.tile([128, 1], F32); nc.sync.dma_start(b3sb[:], bn3b.ap())
            n1wsb = cst.tile([128, 128], F32); nc.sync.dma_start(n1wsb[:], n1w.ap())
            n1gsb = cst.tile([128, 128], F32); nc.sync.dma_start(n1gsb[:], n1g.ap())
            n2wsb = cst.tile([128, 16], F32); nc.sync.dma_start(n2wsb[:], n2w.ap())
            n2gsb = cst.tile([128, 16], F32); nc.sync.dma_start(n2gsb[:], n2g.ap())
            fwsb = cst.tile([16, 1], F32); nc.sync.dma_start(fwsb[:], fw.ap())
            fbsb = cst.tile([16, 1], F32); nc.sync.dma_start(fbsb[:], fbt.ap())
            eps_bn = cst.tile([128, 1], F32); nc.vector.memset(eps_bn[:], BN_EPS)
            eps_nalu = cst.tile([128, 1], F32); nc.vector.memset(eps_nalu[:], NALU_EPS)

            # DRAM scratch: y2 spill + collective bounce buffers
            y2d = drp.tile([n_loc, 2, 128, lout], BF16)
            b_in = [drp.tile([128, 2], F32, name="bin0"),
                    drp.tile([128, 4], F32, name="bin1"),
                    drp.tile([128, 2], F32, name="bin2")]
            b_out = [drp.tile([128, 2], F32, name="bout0"),
                     drp.tile([128, 4], F32, name="bout1"),
                     drp.tile([128, 2], F32, name="bout2")]

            nstats = n_loc * nch * 6

            def dbg_dump(src_ap, width):
                dstats = sml.tile([128, 64], F32, tag="dstats")
                nc.vector.memset(dstats[:], 0.0)
                nc.vector.tensor_copy(dstats[:, 0:width], src_ap)
                nc.sync.dma_start(dbg.ap(), dstats[:])

            def combine_and_allreduce(st_tiles, layer_idx, ngr):
                """st_tiles: per-group stats tiles [128, nstats] holding bn_stats
                triples. Produces global (sum, sumsq) -> scale/shift [128, ngr]."""
                arin = sml.tile([128, 2 * ngr], F32, tag=f"arin{layer_idx}")
                for g in range(ngr):
                    s3 = st_tiles[g].rearrange("p (b t) -> p b t", t=3)
                    counts, means, m2s = s3[:, :, 0], s3[:, :, 1], s3[:, :, 2]
                    nblk = nstats // 3
                    cm = cmbp.tile([128, nblk], F32, tag="cmb")
                    # sum = sum_b count_b * mean_b
                    # (tensor_tensor_reduce faults this runtime; use mult+reduce)
                    nc.vector.tensor_tensor(out=cm[:], in0=counts, in1=means, op=ALU.mult)
                    nc.vector.tensor_reduce(out=arin[:, 2 * g:2 * g + 1], in_=cm[:],
                                            axis=mybir.AxisListType.X, op=ALU.add)
                    # sumsq = sum_b M2_b + sum_b (count_b*mean_b)*mean_b
                    cmm = cmbp.tile([128, nblk], F32, tag="cmb")
                    nc.vector.tensor_tensor(out=cmm[:], in0=cm[:], in1=means, op=ALU.mult)
                    ta = sml.tile([128, 1], F32, tag=f"ta{layer_idx}{g}")
                    nc.vector.tensor_reduce(out=ta[:], in_=cmm[:],
                                            axis=mybir.AxisListType.X, op=ALU.add)
                    tb = sml.tile([128, 1], F32, tag=f"tb{layer_idx}{g}")
                    nc.vector.tensor_reduce(out=tb[:], in_=m2s, axis=mybir.AxisListType.X,
                                            op=ALU.add)
                    nc.vector.tensor_tensor(out=arin[:, 2 * g + 1:2 * g + 2],
                                            in0=ta[:], in1=tb[:], op=ALU.add)
                if stop_after == "AR1a" and layer_idx == 0:
                    dbg_dump(arin[:], 2 * ngr)
                    return None, None
                nc.gpsimd.dma_start(b_in[layer_idx][:], arin[:])
                nc.gpsimd.collective_compute(
                    "AllReduce", ALU.add, replica_groups=[list(range(ncores))],
                    ins=[b_in[layer_idx].opt()], outs=[b_out[layer_idx].opt()])
                gl = sml.tile([128, 2 * ngr], F32, tag=f"gl{layer_idx}")
                nc.sync.dma_start(gl[:], b_out[layer_idx][:])
                if stop_after == "AR1b" and layer_idx == 0:
                    dbg_dump(gl[:], 2 * ngr)
                    return None, None
                # mean/var -> scale = gamma/sqrt(var+eps), shift = beta - mean*scale
                gsb = [g1sb, g2sb, g3sb][layer_idx]
                bsb = [b1sb, b2sb, b3sb][layer_idx]
                mean = sml.tile([128, ngr], F32, tag=f"mean{layer_idx}")
                var = sml.tile([128, ngr], F32, tag=f"var{layer_idx}")
                sd = sml.tile([128, ngr], F32, tag=f"sd{layer_idx}")
                isd = sml.tile([128, ngr], F32, tag=f"isd{layer_idx}")
                scl = sml.tile([128, ngr], F32, tag=f"scl{layer_idx}")
                shf = sml.tile([128, ngr], F32, tag=f"shf{layer_idx}")
                sums = gl.rearrange("p (g t) -> p g t", t=2)
                nc.vector.tensor_scalar_mul(mean[:], sums[:, :, 0], 1.0 / ntot)
                nc.vector.tensor_scalar_mul(var[:], sums[:, :, 1], 1.0 / ntot)
                nc.vector.tensor_tensor(out=sd[:], in0=mean[:], in1=mean[:], op=ALU.mult)
                nc.vector.tensor_tensor(out=var[:], in0=var[:], in1=sd[:], op=ALU.subtract)
                nc.scalar.activation(sd[:], var[:], AF.Sqrt, bias=eps_bn[:], scale=1.0)
                nc.vector.reciprocal(isd[:], sd[:])
                nc.vector.tensor_tensor(out=scl[:], in0=gsb[:, 0:ngr], in1=isd[:], op=ALU.mult)
                nc.vector.tensor_tensor(out=shf[:], in0=mean[:], in1=scl[:], op=ALU.mult)
                nc.vector.tensor_tensor(out=shf[:], in0=bsb[:, 0:ngr], in1=shf[:], op=ALU.subtract)
                return scl, shf

            def emit():
                # ================= Phase A: conv1 + L1 stats =================
                st1 = stp.tile([128, nstats], F32, tag="stats")
                y1 = []
                for i in range(n_loc):
                    t = bigp.tile([128, w], BF16, tag="big", name=f"y1_{i}")
                    y1.append(t)
                    nc.vector.memset(t[:, 0:2], 0.0)
                    nc.vector.memset(t[:, 2 + lout:w], 0.0)
                    for c, (l0, n) in enumerate(CH):
                        rhs = c1p.tile([8, 512], BF16, tag="c1rhs")
                        nc.sync.dma_start(rhs[:, 0:n], win_ap(i, l0, n, 8))
                        ps = (mmp if n > 16 else mm1p).tile(
                            [128, n], F32, tag="mm" if n > 16 else "mm1")
                        nc.tensor.matmul(ps[:], w1sb[:], rhs[:, 0:n], start=True, stop=True)
                        sl = st1[:, (i * nch + c) * 6:(i * nch + c) * 6 + 6]
                        nc.vector.bn_stats(sl, ps[:])
                        if n % 2:
                            nc.vector.memset(
                                st1[:, (i * nch + c) * 6 + 3:(i * nch + c) * 6 + 6], 0.0)
                        nc.scalar.copy(t[:, 2 + l0:2 + l0 + n], ps[:])
                if stop_after == "A":
                    dbg_dump(st1[:, 0:min(64, nstats)], min(64, nstats))
                    return

                scl1, shf1 = combine_and_allreduce([st1], 0, 1)
                if stop_after in ("AR1a", "AR1b"):
                    return
                if stop_after == "AR1":
                    dbg_dump(scl1[:], 1)
                    return

                # ============ Phase B: bn1+relu in place, conv2, spill ============
                st2 = [stp.tile([128, nstats], F32, tag="stats", name="st2a"),
                       stp.tile([128, nstats], F32, tag="stats", name="st2b")]
                for i in range(n_loc):
                    nc.scalar.activation(y1[i][:, 2:2 + lout], y1[i][:, 2:2 + lout],
                                         AF.Relu, bias=shf1[:, 0:1], scale=scl1[:, 0:1])
                    for c, (l0, n) in enumerate(CH):
                        for g in groups:
                            ps = (mmp if n > 16 else mm1p).tile(
                                [128, n], F32, tag="mm" if n > 16 else "mm1")
                            for k in range(5):
                                nc.tensor.matmul(
                                    ps[:], w2sb[:, (k * 2 + g) * 128:(k * 2 + g + 1) * 128],
                                    y1[i][:, l0 + k:l0 + k + n],
                                    start=(k == 0), stop=(k == 4))
                            sl = st2[g][:, (i * nch + c) * 6:(i * nch + c) * 6 + 6]
                            nc.vector.bn_stats(sl, ps[:])
                            if n % 2:
                                nc.vector.memset(
                                    st2[g][:, (i * nch + c) * 6 + 3:(i * nch + c) * 6 + 6], 0.0)
                            sp = spop.tile([128, 512], BF16, tag="spo")
                            nc.scalar.copy(sp[:, 0:n], ps[:])
                            nc.sync.dma_start(y2d[i, g, :, l0:l0 + n], sp[:, 0:n])
                if stop_after == "B":
                    dbg_dump(st2[0][:, 0:min(64, nstats)], min(64, nstats))
                    return

                scl2, shf2 = combine_and_allreduce(st2, 1, 2)

                # ============ Phase C: bn2+relu, conv3, y3 into big pool ============
                st3 = stp.tile([128, nstats], F32, tag="stats")
                y3 = []
                for i in range(n_loc):
                    x2 = [x2p.tile([128, w], BF16, tag="x2a", name=f"x2a{i}"),
                          x2p.tile([128, w], BF16, tag="x2b", name=f"x2b{i}")]
                    for g in groups:
                        nc.vector.memset(x2[g][:, 0:1], 0.0)
                        nc.vector.memset(x2[g][:, 1 + lout:w], 0.0)
                        for c, (l0, n) in enumerate(CH):
                            sp = spip.tile([128, 512], BF16, tag="spi")
                            nc.sync.dma_start(sp[:, 0:n], y2d[i, g, :, l0:l0 + n])
                            nc.scalar.activation(x2[g][:, 1 + l0:1 + l0 + n], sp[:, 0:n],
                                                 AF.Relu, bias=shf2[:, g:g + 1],
                                                 scale=scl2[:, g:g + 1])
                    t = bigp.tile([128, w], BF16, tag="big", name=f"y3_{i}")
                    y3.append(t)
                    for c, (l0, n) in enumerate(CH):
                        ps = (mmp if n > 16 else mm1p).tile(
                            [128, n], F32, tag="mm" if n > 16 else "mm1")
                        for kt in range(2):
                            for k in range(3):
                                nc.tensor.matmul(
                                    ps[:], w3sb[:, (kt * 3 + k) * 128:(kt * 3 + k + 1) * 128],
                                    x2[kt][:, l0 + k:l0 + k + n],
                                    start=(kt == 0 and k == 0), stop=(kt == 1 and k == 2))
                        sl = st3[:, (i * nch + c) * 6:(i * nch + c) * 6 + 6]
                        nc.vector.bn_stats(sl, ps[:])
                        if n % 2:
                            nc.vector.memset(
                                st3[:, (i * nch + c) * 6 + 3:(i * nch + c) * 6 + 6], 0.0)
                        nc.vector.tensor_copy(t[:, 2 + l0:2 + l0 + n], ps[:])
                if stop_after == "C":
                    dbg_dump(st3[:, 0:min(64, nstats)], min(64, nstats))
                    return

                scl3, shf3 = combine_and_allreduce([st3], 2, 1)

                # ============ Phase D: bn3+relu+mean -> featT ============
                featT = sml.tile([128, n_loc], F32, tag="featT")
                for i in range(n_loc):
                    acc = accp.tile([128, 16], F32, tag="accp")
                    for c, (l0, n) in enumerate(CH):
                        dmp = dmpp.tile([128, 512], BF16, tag="dump")
                        nc.scalar.activation(dmp[:, 0:n], y3[i][:, 2 + l0:2 + l0 + n],
                                             AF.Relu, bias=shf3[:, 0:1], scale=scl3[:, 0:1],
                                             accum_out=acc[:, c:c + 1])
                    fs = sml.tile([128, 1], F32, tag="fs")
                    nc.vector.tensor_reduce(out=fs[:], in_=acc[:, 0:nch],
                                            axis=mybir.AxisListType.X, op=ALU.add)
                    nc.vector.tensor_scalar_mul(featT[:, i:i + 1], fs[:], 1.0 / lout)
                if stop_after == "D":
                    dbg_dump(featT[:], n_loc)
                    return

                # ============ Phase E: NALU x2 + final linear (fp32) ============
                def nalu(xT, wT, gT, m_out):
                    """xT [128, n_loc] in; returns hT [m_out, n_loc]."""
                    aps = mm1p.tile([m_out, n_loc], F32, tag="mm1")
                    nc.tensor.matmul(aps[:], wT[:, 0:m_out], xT[:], start=True, stop=True)
                    gps = mm1p.tile([m_out, n_loc], F32, tag="mm1")
                    nc.tensor.matmul(gps[:], gT[:, 0:m_out], xT[:], start=True, stop=True)
                    gsb_ = sml.tile([m_out, n_loc], F32, tag=f"gsb{m_out}")
                    nc.scalar.activation(gsb_[:], gps[:], AF.Sigmoid)
                    ab = sml.tile([128, n_loc], F32, tag=f"ab{m_out}")
                    nc.scalar.activation(ab[:], xT[:], AF.Abs)
                    ln = sml.tile([128, n_loc], F32, tag=f"ln{m_out}")
                    nc.scalar.activation(ln[:], ab[:], AF.Ln, bias=eps_nalu[:], scale=1.0)
                    mps = mm1p.tile([m_out, n_loc], F32, tag="mm1")
                    nc.tensor.matmul(mps[:], wT[:, 0:m_out], ln[:], start=True, stop=True)
                    mt = sml.tile([m_out, n_loc], F32, tag=f"mt{m_out}")
                    nc.scalar.activation(mt[:], mps[:], AF.Exp)
                    d = sml.tile([m_out, n_loc], F32, tag=f"d{m_out}")
                    nc.vector.tensor_tensor(out=d[:], in0=aps[:], in1=mt[:], op=ALU.subtract)
                    nc.vector.tensor_tensor(out=d[:], in0=gsb_[:], in1=d[:], op=ALU.mult)
                    h = sml.tile([m_out, n_loc], F32, tag=f"h{m_out}")
                    nc.vector.tensor_tensor(out=h[:], in0=d[:], in1=mt[:], op=ALU.add)
                    return h

                h1 = nalu(featT, n1wsb, n1gsb, 128)
                h2 = nalu(h1, n2wsb, n2gsb, 16)
                fin = mm1p.tile([n_loc, 1], F32, tag="mm1")
                nc.tensor.matmul(fin[:], h2[:, 0:n_loc], fwsb[:], start=True, stop=True)
                osb = sml.tile([n_loc, 1], F32, tag="osb")
                nc.scalar.activation(osb[:], fin[:], AF.Identity, bias=fbsb[0:n_loc, :],
                                     scale=1.0)
                nc.sync.dma_start(outd.ap(), osb[:])

            emit()

    nc.compile()
    return nc


def prep_inputs(inputs, conv1_w, conv2_w, conv3_w, bn1_g, bn1_b, bn2_g, bn2_b,
                bn3_g, bn3_b, nalu1_What, nalu1_Mhat, nalu1_G, nalu2_What,
                nalu2_Mhat, nalu2_G, final_w, final_b, ncores, n_loc):
    """Host-side layout prep: pad+cast input, transpose weights into lhsT layouts."""
    bf = ml_dtypes.bfloat16
    f32 = np.float32
    xpad = np.pad(np.asarray(inputs, f32), ((0, 0), (4, 4))).astype(bf)

    w1t = np.ascontiguousarray(np.asarray(conv1_w, f32)[:, 0, :].T).astype(bf)
    t2 = np.asarray(conv2_w, f32).reshape(2, 128, 128, 5).transpose(2, 3, 0, 1)
    w2t = np.ascontiguousarray(t2.reshape(128, 10 * 128)).astype(bf)
    t3 = np.asarray(conv3_w, f32).reshape(128, 2, 128, 3).transpose(2, 1, 3, 0)
    w3t = np.ascontiguousarray(t3.reshape(128, 6 * 128)).astype(bf)

    def sig(x):
        return 1.0 / (1.0 + np.exp(-x.astype(np.float64)))

    w1 = (np.tanh(np.asarray(nalu1_What, np.float64)) * sig(np.asarray(nalu1_Mhat))).astype(f32)
    w2 = (np.tanh(np.asarray(nalu2_What, np.float64)) * sig(np.asarray(nalu2_Mhat))).astype(f32)
    common = {
        "w1t": w1t, "w2t": w2t, "w3t": w3t,
        "bn1g": np.asarray(bn1_g, f32)[:, None], "bn1b": np.asarray(bn1_b, f32)[:, None],
        "bn2g": np.ascontiguousarray(np.asarray(bn2_g, f32).reshape(2, 128).T),
        "bn2b": np.ascontiguousarray(np.asarray(bn2_b, f32).reshape(2, 128).T),
        "bn3g": np.asarray(bn3_g, f32)[:, None], "bn3b": np.asarray(bn3_b, f32)[:, None],
        "n1w": np.ascontiguousarray(w1.T), "n1g": np.ascontiguousarray(np.asarray(nalu1_G, f32).T),
        "n2w": np.ascontiguousarray(w2.T), "n2g": np.ascontiguousarray(np.asarray(nalu2_G, f32).T),
        "fw": np.ascontiguousarray(np.asarray(final_w, f32).T),
        "fbt": np.full((16, 1), np.asarray(final_b, f32)[0], f32),
    }
    in_maps = []
    for i in range(ncores):
        m = dict(common)
        m["xpad"] = np.ascontiguousarray(xpad[i * n_loc:(i + 1) * n_loc])
        in_maps.append(m)
    return in_maps


_CACHED_NC = None


def kernel(**inputs):
    global _CACHED_NC
    n_loc = B // NCORES
    if _CACHED_NC is None:
        _CACHED_NC = build_nc(NCORES, n_loc, L_IN)
    in_maps = prep_inputs(
        inputs["inputs"], inputs["conv1_w"], inputs["conv2_w"], inputs["conv3_w"],
        inputs["bn1_g"], inputs["bn1_b"], inputs["bn2_g"], inputs["bn2_b"],
        inputs["bn3_g"], inputs["bn3_b"],
        inputs["nalu1_What"], inputs["nalu1_Mhat"], inputs["nalu1_G"],
        inputs["nalu2_What"], inputs["nalu2_Mhat"], inputs["nalu2_G"],
        inputs["final_w"], inputs["final_b"], NCORES, n_loc)
    res = bass_utils.run_bass_kernel_spmd(_CACHED_NC, in_maps,
                                          core_ids=list(range(NCORES)))
    return np.concatenate([res.results[i]["out"] for i in range(NCORES)], axis=0)
